# revision 14
# baseline (speedup 1.0000x reference)
"""AnomalyTransformer forward pass on 8 Trainium2 NeuronCores.

Data-parallel over batch: each core processes 32 of the 256 batch elements
through the full 3-layer transformer.

Precision strategy: the residual stream h and all projection weights on
the q/k path run in float32r (TF32-like TensorEngine mode, full throughput
at moving-dim >= 256); q/k chunk tiles, the value path and attention
output run in bf16. Softmax logits reach +-38 in layer 3, so an all-bf16
kernel amplifies rounding to ~1.5e-2 relative error; this mix lands at
~6e-3 against the f32 reference.

Layout strategy: the residual stream h is feature-major ([D, tokens], D
split over 4 partition-tiles of 128). Attention uses the scoresT
orientation (scoresT = khT.T @ qhT -> [l_k, l_q]) so softmax normalization
folds into the attention-value matmul via an appended ones-column on V
(column 64 of each head's 65-wide slot accumulates sum(exp)); the
per-token reciprocal is then a per-partition scalar multiply. One
[100, 512] PE-transpose per batch element brings the attention output back
to feature-major for the Wo projection.

Pipeline strategy (what makes PE stay busy):
  - V-projection PSUM is double-buffered (2 banks) so the per-batch V
    matmuls overlap the DVE copies into SBUF.
  - The per-batch attention loop is software-pipelined on PE:
    scores(b) -> vproj(b+1) -> transpose(b-1) -> attnV(b), hiding the
    ACT exp latency and the DVE normalize latency behind PE work.
  - The last batch's transpose and the Wo projection of chunk g are
    deferred into chunk g+1 (independent token slices), hiding the
    normalize/copy chain behind a full projection block.
  - Copies are balanced across ACT and DVE (GPSIMD has no PSUM port and
    every post-matmul op reads PSUM, so it cannot help).
  - FFN runs W1 one chunk ahead of W2, with the residual add split
    between DVE adds and PE identity-matmul accumulation + ACT copyback;
    the output projection is folded into the last layer's FFN loop.
  - The circular-conv token embedding ships x once ([55, 102] padded
    windows per batch) and applies the k=3 unfold as 3 shifted-view
    matmuls, cutting input DMA 3x.

The sigma/prior branch of the reference is dead code (never feeds the
output) and is skipped. Biases in the reference are all zeros and are
skipped.
"""

import sys
import os
for _p in ("/opt/trn_rl_repo", "/root/.axon_site/_ro/trn_rl_repo"):
    if os.path.isdir(_p) and _p not in sys.path:
        sys.path.insert(0, _p)

import math
import numpy as np
import ml_dtypes

import concourse.bass as bass
import concourse.tile as tile
from concourse import mybir
from concourse.bass_utils import run_bass_kernel_spmd
from contextlib import ExitStack

BF16 = mybir.dt.bfloat16
F32 = mybir.dt.float32
F32R = mybir.dt.float32r
AF = mybir.ActivationFunctionType
OP = mybir.AluOpType

# model dims
B, L, C, D, H, NL, DFF = 256, 100, 55, 512, 8, 3, 64
DK = D // H                      # 64
NCORES = 8
BL = B // NCORES                 # 32 batches per core
TOK = BL * L                     # 3200 tokens per core
TCH = 400                        # token chunk (4 batches)
NT = TOK // TCH                  # 8 chunks
CB = TCH // L                    # 4 batches per chunk
KT = D // 128                    # 4 contraction tiles
LP = L + 2                       # padded window for the k=3 circular conv


_NOSTRUCT = ("InstDrain", "InstNoOp", "InstEventSemaphore", "InstHalt")


def _legalize_waits(nc, maxw=1):
    """This container's walrus caps sync-waits at 1 per instruction; move
    extra waits onto preceding same-engine NOPs (one wait each)."""
    cnt = [0]
    for f in nc.m.functions:
        for blk in f.blocks:
            newlist = []
            changed = False
            for ins in blk.instructions:
                si = getattr(ins, "sync_info", None)
                lim = maxw
                if si is not None and si.on_wait and len(si.on_wait) > lim:
                    waits = list(si.on_wait)
                    extra, keep = waits[:-lim], waits[-lim:]
                    for i in range(0, len(extra), 1):
                        cnt[0] += 1
                        nop = mybir.InstNoOp(
                            name=f"I-ws-{cnt[0]}", ins=[], outs=[], engine=ins.engine
                        )
                        nop.sync_info = mybir.SyncInfo(
                            on_wait=extra[i:i + 1], on_update=[]
                        )
                        newlist.append(nop)
                    ins.sync_info = mybir.SyncInfo(
                        on_wait=keep, on_update=list(si.on_update)
                    )
                    changed = True
                newlist.append(ins)
            if changed:
                blk.instructions = newlist
    return nc


def _offset_ap(ap, extra_offset, dims):
    """AP at ap.offset + extra_offset (elements) with free dims `dims`
    ([[step, count], ...]), keeping ap's partition dim."""
    return bass.AP(tensor=ap.tensor, offset=ap.offset + extra_offset,
                   ap=[list(ap.ap[0])] + [list(d) for d in dims])


def build_nc():
    nc = bass.Bass()

    # ---- DRAM parameters (host-prepped) ----
    # xpad[c, t, b, l] : per-chunk circular-padded windows, [55, NT*CB*102]
    xpad_d = nc.declare_dram_parameter("xpad", [C, NT, CB, LP], F32R, isOutput=False)
    wemb_d = nc.declare_dram_parameter("wemb", [C, 3, D], F32R, isOutput=False)
    pe_d = nc.declare_dram_parameter("pe", [128, KT, L], F32, isOutput=False)
    wq_d = nc.declare_dram_parameter("wq", [128, NL, KT, D], F32R, isOutput=False)
    wk_d = nc.declare_dram_parameter("wk", [128, NL, KT, D], F32R, isOutput=False)
    wv_d = nc.declare_dram_parameter("wv", [128, NL, KT, D], F32R, isOutput=False)
    wo_d = nc.declare_dram_parameter("wo", [128, NL, KT, D], BF16, isOutput=False)
    w1_d = nc.declare_dram_parameter("w1", [128, NL, KT, DFF], F32R, isOutput=False)
    w2_d = nc.declare_dram_parameter("w2", [DFF, NL, KT, 128], F32R, isOutput=False)
    wout_d = nc.declare_dram_parameter("wout", [128, KT, C], F32R, isOutput=False)
    identb_d = nc.declare_dram_parameter("identb", [128, 128], BF16, isOutput=False)
    identr_d = nc.declare_dram_parameter("identr", [128, 128], F32R, isOutput=False)
    out_d = nc.declare_dram_parameter("out", [C, TOK], F32, isOutput=True)

    with tile.TileContext(nc) as tc, ExitStack() as stk:
        tc.race_detector_enabled = False
        singles = stk.enter_context(tc.tile_pool(name="singles", bufs=1))
        wp = stk.enter_context(tc.tile_pool(name="wp", bufs=2))
        xp = stk.enter_context(tc.tile_pool(name="xp", bufs=3))
        qp = stk.enter_context(tc.tile_pool(name="qp", bufs=2))
        kp = stk.enter_context(tc.tile_pool(name="kp", bufs=2))
        vp = stk.enter_context(tc.tile_pool(name="vp", bufs=2))
        expp = stk.enter_context(tc.tile_pool(name="expp", bufs=3))
        op_ = stk.enter_context(tc.tile_pool(name="op", bufs=3))
        rp = stk.enter_context(tc.tile_pool(name="rp", bufs=3))
        otp = stk.enter_context(tc.tile_pool(name="otp", bufs=2))
        yp = stk.enter_context(tc.tile_pool(name="yp", bufs=2))
        outp = stk.enter_context(tc.tile_pool(name="outp", bufs=2))
        # psum pools (8 banks total); the FFN W1 accumulator borrows the
        # attention-idle vpp pool
        pp = stk.enter_context(tc.tile_pool(name="pp", bufs=3, space="PSUM"))
        scp = stk.enter_context(tc.tile_pool(name="scp", bufs=3, space="PSUM"))
        vpp = stk.enter_context(tc.tile_pool(name="vpp", bufs=2, space="PSUM"))

        # ---- persistent SBUF ----
        wemb_sb = singles.tile([C, 3, D], F32R)
        pe_sb = singles.tile([128, KT, L], F32)
        wout_sb = singles.tile([128, KT, C], F32R)
        ident_b = singles.tile([128, 128], BF16)
        ident_r = singles.tile([128, 128], F32R)
        h_sb = [singles.tile([128, TOK], F32R, name=f"h{k}") for k in range(KT)]
        xc = [xp.tile([C, CB, LP], F32R, name=f"xc{t}", tag="xc") for t in range(NT)]

        # DMA priority order: embedding weights + first chunks of x first,
        # then the layer-0 projection weights (interleaved with remaining x
        # chunks), then late-needed singles.
        nc.sync.dma_start(out=wemb_sb[:], in_=wemb_d[:])
        nc.sync.dma_start(out=xc[0][:], in_=xpad_d[:, 0])
        nc.sync.dma_start(out=pe_sb[:], in_=pe_d[:])
        nc.sync.dma_start(out=xc[1][:], in_=xpad_d[:, 1])
        nc.sync.dma_start(out=ident_b[:], in_=identb_d[:])
        nc.sync.dma_start(out=ident_r[:], in_=identr_d[:])
        for t in range(2, NT):
            nc.sync.dma_start(out=xc[t][:], in_=xpad_d[:, t])

        # ---- token embedding: circular conv as 3 shifted matmuls, + pe ----
        for t in range(NT):
            tsl = slice(t * TCH, (t + 1) * TCH)
            for m in range(KT):
                ps = pp.tile([128, 512], F32, tag="pp")
                for d in range(3):
                    # moving: xc[t][:, b, d : d+100] for the 4 batches
                    mv = _offset_ap(xc[t][:, 0, 0], d, [[LP, CB], [1, L]])
                    nc.tensor.matmul(ps[:, :TCH], wemb_sb[:, d, m * 128:(m + 1) * 128],
                                     mv, start=(d == 0), stop=(d == 2))
                pe_b = _offset_ap(pe_sb[:, m, :], 0, [[0, CB], [1, L]])
                nc.vector.tensor_tensor(
                    h_sb[m][:, tsl].rearrange("p (b x) -> p b x", x=L),
                    ps[:, :TCH].rearrange("p (b x) -> p b x", x=L),
                    pe_b, op=OP.add)

        nc.sync.dma_start(out=wout_sb[:], in_=wout_d[:])

        # ---- transformer layers ----
        n_layer_passes = int(os.environ.get("ANOM_LAYERS", str(NL)))
        for lp_i in range(n_layer_passes):
            l = lp_i % NL
            last_layer = lp_i == n_layer_passes - 1
            wq_l = wp.tile([128, KT, D], F32R, tag="wq")
            wk_l = wp.tile([128, KT, D], F32R, tag="wk")
            wv_l = wp.tile([128, KT, D], F32R, tag="wv")
            wo_l = wp.tile([128, KT, D], BF16, tag="wo")
            w1_l = wp.tile([128, KT, DFF], F32R, tag="w1")
            w2_l = wp.tile([DFF, KT, 128], F32R, tag="w2")
            for dst, src in ((wq_l, wq_d), (wk_l, wk_d), (wv_l, wv_d),
                             (wo_l, wo_d), (w1_l, w1_d), (w2_l, w2_d)):
                nc.sync.dma_start(out=dst[:], in_=src[:, l])

            # ---- attention pass over chunks ----
            # pend = (g, ot_all, transpose_o, 3): the last batch's transpose
            # and the whole Wo projection of chunk g are deferred into chunk
            # g+1 so the normalize/copy chain hides behind QKproj(g+1).
            pend = None

            def do_wo(g, ot_all):
                gsl = slice(g * TCH, (g + 1) * TCH)
                for m in range(KT):
                    msl = slice(m * 128, (m + 1) * 128)
                    ps = pp.tile([128, 512], F32, tag="pp")
                    for k in range(KT):
                        nc.tensor.matmul(ps[:, :TCH], wo_l[:, k, msl],
                                         ot_all[:, k, :],
                                         start=(k == 0), stop=(k == KT - 1))
                    nc.vector.tensor_tensor(h_sb[m][:, gsl], ps[:, :TCH],
                                            h_sb[m][:, gsl], op=OP.add)

            def flush_pend():
                g_p, ot_p, tr_p = pend
                tr_p(CB - 1)
                do_wo(g_p, ot_p)

            for g in range(NT):
                gsl = slice(g * TCH, (g + 1) * TCH)

                # V tile, token-major per batch (65-stride heads,
                # col 65h+64 = 1 for the softmax-sum trick)
                v_t = vp.tile([128, CB, 8 * 65], BF16, tag="v")
                nc.vector.memset(
                    v_t[:L, :, :].rearrange(
                        "p b (h x) -> p b h x", x=65)[:, :, :, 64:65], 1.0)

                def vproj_mm(bi):
                    b = g * CB + bi
                    bsl = slice(b * L, (b + 1) * L)
                    ps = vpp.tile([128, 512], F32, tag="vps", name="vps")
                    for k in range(KT):
                        nc.tensor.matmul(ps[:L, :], h_sb[k][:, bsl],
                                         wv_l[:, k, :],
                                         start=(k == 0), stop=(k == KT - 1))
                    return ps

                def vcopy(bi, ps, eng):
                    eng(v_t[:L, bi, :].rearrange(
                            "p (h x) -> p h x", x=65)[:, :, :64],
                        ps[:L, :].rearrange("p (h x) -> p h x", x=64))

                # batch-0 V projection first: PE filler that covers the
                # chunk-boundary wait on the previous batch's normalize
                # (the first QK psum slot is freed by it)
                vps0 = vproj_mm(0)
                vcopy(0, vps0, nc.vector.tensor_copy)

                # Q/K projections for this chunk, feature-major [D, TCH]
                qc = [qp.tile([128, TCH], BF16, name=f"qc{m}", tag=f"qc{m}")
                      for m in range(KT)]
                kc = [kp.tile([128, TCH], BF16, name=f"kc{m}", tag=f"kc{m}")
                      for m in range(KT)]
                for m in range(KT):
                    msl = slice(m * 128, (m + 1) * 128)
                    ps = pp.tile([128, 512], F32, tag="pp")
                    for k in range(KT):
                        nc.tensor.matmul(ps[:, :TCH], wq_l[:, k, msl],
                                         h_sb[k][:, gsl],
                                         start=(k == 0), stop=(k == KT - 1))
                    nc.scalar.copy(qc[m][:], ps[:, :TCH])
                for m in range(KT):
                    msl = slice(m * 128, (m + 1) * 128)
                    ps = pp.tile([128, 512], F32, tag="pp")
                    for k in range(KT):
                        nc.tensor.matmul(ps[:, :TCH], wk_l[:, k, msl],
                                         h_sb[k][:, gsl],
                                         start=(k == 0), stop=(k == KT - 1))
                    nc.scalar.copy(kc[m][:], ps[:, :TCH])

                # previous chunk's deferred transpose + Wo projection: keeps
                # PE fed while this chunk's attention pipeline fills, and
                # gives the previous chunk's normalize/copy chain time.
                if pend is not None:
                    flush_pend()
                    pend = None

                ot_all = otp.tile([128, KT, TCH], BF16, tag="ot")
                o_ts = [None] * CB

                def transpose_o(bi):
                    tp = pp.tile([128, 1024], BF16, tag="pp", name="tp")
                    for m in range(KT):
                        nc.tensor.transpose(tp[:, m * L:(m + 1) * L],
                                            o_ts[bi][:L, m * 128:(m + 1) * 128],
                                            ident_b[:L, :L])
                    nc.vector.tensor_copy(
                        _offset_ap(ot_all[:, :, :], bi * L, [[TCH, KT], [1, L]]),
                        tp[:, :KT * L].rearrange("p (m x) -> p m x", x=L))

                for bi in range(CB):
                    csl = slice(bi * L, (bi + 1) * L)
                    # scoresT for 8 heads: even heads -> scA, odd -> scB
                    # (different PE row groups must write different PSUM
                    # banks). Evens first so exp(scA) starts early.
                    scA = scp.tile([128, 512], F32, tag="sc")
                    scB = scp.tile([128, 512], F32, tag="sc")
                    for hh in (0, 2, 4, 6, 1, 3, 5, 7):
                        kt_i, base = divmod(hh * DK, 128)
                        sc = scA if hh % 2 == 0 else scB
                        col = (hh // 2) * 128
                        nc.tensor.matmul(sc[:L, col:col + L],
                                         kc[kt_i][base:base + DK, csl],
                                         qc[kt_i][base:base + DK, csl],
                                         start=True, stop=True)
                    vps_n = vproj_mm(bi + 1) if bi + 1 < CB else None
                    exp_t = expp.tile([128, 8 * L], BF16, tag="exp")
                    # exp; head hh lands at exp_t cols hh*L
                    nc.scalar.activation(
                        exp_t[:L, :].rearrange("p (h x) -> p h x", x=2 * L)[:, :, :L],
                        scA[:L, :].rearrange("p (h x) -> p h x", x=128)[:, :, :L],
                        AF.Exp)
                    nc.scalar.activation(
                        _offset_ap(exp_t[:L, :], L, [[2 * L, 4], [1, L]]),
                        scB[:L, :].rearrange("p (h x) -> p h x", x=128)[:, :, :L],
                        AF.Exp)
                    # V copy for the next batch: ACT for odd batches, emitted
                    # after exp(bi) so it never delays the exp attnV waits on
                    if vps_n is not None and (bi + 1) % 2:
                        vcopy(bi + 1, vps_n, nc.scalar.copy)
                    # oU = expST.T @ [v | 1]  (token-major, col 64 = sum(exp))
                    # even heads, half the previous transpose, odd heads,
                    # rest of the transpose: PE filler that covers the expB
                    # and normalize latencies
                    ouA = pp.tile([128, 512], F32, tag="pp", name="ouA")
                    tp = (pp.tile([128, 1024], BF16, tag="pp", name="tp")
                          if bi > 0 else None)
                    ouB = pp.tile([128, 512], F32, tag="pp", name="ouB")
                    for hh in (0, 2, 4, 6):
                        nc.tensor.matmul(ouA[:L, (hh // 2) * 128:(hh // 2) * 128 + 65],
                                         exp_t[:L, hh * L:(hh + 1) * L],
                                         v_t[:L, bi, hh * 65:(hh + 1) * 65],
                                         start=True, stop=True)
                    if bi > 0:
                        for m in (0, 1):
                            nc.tensor.transpose(
                                tp[:, m * L:(m + 1) * L],
                                o_ts[bi - 1][:L, m * 128:(m + 1) * 128],
                                ident_b[:L, :L])
                    for hh in (1, 3, 5, 7):
                        nc.tensor.matmul(ouB[:L, (hh // 2) * 128:(hh // 2) * 128 + 65],
                                         exp_t[:L, hh * L:(hh + 1) * L],
                                         v_t[:L, bi, hh * 65:(hh + 1) * 65],
                                         start=True, stop=True)
                    if bi > 0:
                        for m in (2, 3):
                            nc.tensor.transpose(
                                tp[:, m * L:(m + 1) * L],
                                o_ts[bi - 1][:L, m * 128:(m + 1) * 128],
                                ident_b[:L, :L])
                    r_t = rp.tile([128, 8], F32, tag="r")
                    o_t = op_.tile([128, D], BF16, tag="o")
                    for i, ou in enumerate((ouA, ouB)):
                        nc.vector.reciprocal(
                            r_t[:L, i * 4:(i + 1) * 4],
                            ou[:L, :].rearrange(
                                "p (h x) -> p h x", x=128)[:, :, 64:65])
                        nc.vector.tensor_tensor(
                            o_t[:L, i * 256:(i + 1) * 256].rearrange(
                                "p (h x) -> p h x", x=64),
                            ou[:L, :].rearrange(
                                "p (h x) -> p h x", x=128)[:, :, :64],
                            r_t[:L, i * 4:(i + 1) * 4].rearrange(
                                "p (h x) -> p h x", x=1).broadcast_to([L, 4, 64]),
                            op=OP.mult)
                    # ot copy and DVE vcopy after the norms: keeps the
                    # normalize chain at the DVE queue head
                    if bi > 0:
                        nc.vector.tensor_copy(
                            _offset_ap(ot_all[:, :, :], (bi - 1) * L,
                                       [[TCH, KT], [1, L]]),
                            tp[:, :KT * L].rearrange("p (m x) -> p m x", x=L))
                    if vps_n is not None and (bi + 1) % 2 == 0:
                        vcopy(bi + 1, vps_n, nc.vector.tensor_copy)
                    o_ts[bi] = o_t
                pend = (g, ot_all, transpose_o)
            flush_pend()

            # ---- FFN pass (W1 runs one chunk ahead of W2) ----
            ps1s = [None] * NT

            def ffn1(g):
                gsl = slice(g * TCH, (g + 1) * TCH)
                ps1 = vpp.tile([128, 512], F32, tag="vps", name="ps1")
                for k in range(KT):
                    nc.tensor.matmul(ps1[:DFF, :TCH], w1_l[:, k, :],
                                     h_sb[k][:, gsl],
                                     start=(k == 0), stop=(k == KT - 1))
                y_t = yp.tile([DFF, TCH], F32R, tag="y")
                nc.scalar.activation(y_t[:, :], ps1[:DFF, :TCH], AF.Gelu)
                return y_t

            def outproj(g):
                tsl = slice(g * TCH, (g + 1) * TCH)
                ps = pp.tile([128, 512], F32, tag="pp")
                for k in range(KT):
                    nc.tensor.matmul(ps[:C, :TCH], wout_sb[:, k, :],
                                     h_sb[k][:, tsl],
                                     start=(k == 0), stop=(k == KT - 1))
                o_f = outp.tile([128, TCH], F32, tag="outc")
                nc.scalar.copy(o_f[:C, :], ps[:C, :TCH])
                nc.sync.dma_start(out=out_d[:, tsl], in_=o_f[:C, :])

            ys = [None] * NT
            ys[0] = ffn1(0)
            for g in range(NT):
                gsl = slice(g * TCH, (g + 1) * TCH)
                if g + 1 < NT:
                    ys[g + 1] = ffn1(g + 1)
                for m in range(KT):
                    ps2 = pp.tile([128, 512], F32, tag="pp", name="ps2")
                    if m < 2:
                        nc.tensor.matmul(ps2[:, :TCH], w2_l[:, m, :], ys[g][:, :],
                                         start=True, stop=True)
                        nc.vector.tensor_tensor(h_sb[m][:, gsl], ps2[:, :TCH],
                                                h_sb[m][:, gsl], op=OP.add)
                    else:
                        # residual folded into PSUM as an identity matmul;
                        # ACT copies back (balances DVE vs ACT in this phase)
                        nc.tensor.matmul(ps2[:, :TCH], w2_l[:, m, :], ys[g][:, :],
                                         start=True, stop=False)
                        nc.tensor.matmul(ps2[:, :TCH], ident_r[:],
                                         h_sb[m][:, gsl],
                                         start=False, stop=True)
                        nc.scalar.copy(h_sb[m][:, gsl], ps2[:, :TCH])
                if last_layer and g >= 1:
                    outproj(g - 1)
            if last_layer:
                outproj(NT - 1)

    return _legalize_waits(nc)


def _bf(a):
    return np.ascontiguousarray(a).astype(ml_dtypes.bfloat16)


def _r32(a):
    """Round to the reduced-dtype grid (f32r: 10 explicit mantissa bits)."""
    a = np.ascontiguousarray(a, np.float32)
    u = a.view(np.uint32).copy()
    u = (u + 0x1000) & 0xFFFFE000
    return u.view(np.float32)


# o features are written evens-first (heads 0,2,4,6 then 1,3,5,7); Wo's
# input-feature rows are permuted to match.
_PERM_DIN = np.concatenate([np.arange(h * DK, (h + 1) * DK)
                            for h in (0, 2, 4, 6, 1, 3, 5, 7)])


def prep_weights(tok_w, pe, Wq, Wk, Wv, Wo, W1, W2, proj_w):
    """Host-side weight reorganization (shared across cores)."""
    scale = 1.0 / math.sqrt(DK)
    # conv as 3 shifted matmuls: wemb[c, d, o] = tok_w[o, c, d]
    wemb = np.ascontiguousarray(np.transpose(tok_w, (1, 2, 0)))  # [C, 3, D]
    # projection weights as lhsT tiles: w[p, l, k, j] = W[l, j, 128k + p]
    def proj_lhsT(W):  # [NL, D_out, D_in] -> [128, NL, KT, D_out]
        return np.ascontiguousarray(
            np.transpose(W, (2, 0, 1)).reshape(KT, 128, NL, W.shape[1])
            .transpose(1, 2, 0, 3))
    eye = np.eye(128, dtype=np.float32)
    m = {
        "identb": _bf(eye), "identr": _r32(eye),
        "wemb": _r32(wemb),
        "pe": np.ascontiguousarray(
            np.ascontiguousarray(pe.T).reshape(KT, 128, L).transpose(1, 0, 2)),
        "wq": _r32(proj_lhsT(Wq * scale)),
        "wk": _r32(proj_lhsT(Wk)),
        "wv": _r32(proj_lhsT(Wv)),
        "wo": _bf(proj_lhsT(Wo[:, :, _PERM_DIN])),
        "w1": _r32(proj_lhsT(W1)),
        # w2[p, l, m, j] = W2[l, 128m + j, p]   (p over DFF=64)
        "w2": _r32(np.transpose(W2, (2, 0, 1)).reshape(DFF, NL, KT, 128)),
        # wout[p, k, j] = proj_w[j, 128k + p]
        "wout": _r32(np.ascontiguousarray(proj_w.T).reshape(KT, 128, C)
                     .transpose(1, 0, 2)),
    }
    return m


def prep_xpad(xs):
    """Per-core input: xs [BL, L, C] -> feature-major circular-padded
    windows [C, NT, CB, L+2]."""
    xt = np.transpose(xs, (2, 0, 1))                     # [C, BL, L]
    xpad = np.empty((C, BL, LP), np.float32)
    xpad[:, :, 1:L + 1] = xt
    xpad[:, :, 0] = xt[:, :, L - 1]
    xpad[:, :, L + 1] = xt[:, :, 0]
    return _r32(xpad.reshape(C, NT, CB, LP))


_NC_CACHE = {}


def get_nc():
    if "nc" not in _NC_CACHE:
        _NC_CACHE["nc"] = build_nc()
    return _NC_CACHE["nc"]


def make_in_maps(inputs):
    x = np.asarray(inputs["x"], np.float32)
    wm = prep_weights(np.asarray(inputs["tok_w"], np.float32),
                      np.asarray(inputs["pe"], np.float32),
                      np.asarray(inputs["Wq"], np.float32),
                      np.asarray(inputs["Wk"], np.float32),
                      np.asarray(inputs["Wv"], np.float32),
                      np.asarray(inputs["Wo"], np.float32),
                      np.asarray(inputs["W1"], np.float32),
                      np.asarray(inputs["W2"], np.float32),
                      np.asarray(inputs["proj_w"], np.float32))
    in_maps = []
    for c in range(NCORES):
        in_maps.append({**wm, "xpad": prep_xpad(x[c * BL:(c + 1) * BL])})
    return in_maps


def assemble_out(results):
    # per-core out [C, TOK] feature-major -> [B, L, C]
    outs = [np.asarray(r["out"], np.float32).reshape(C, BL, L).transpose(1, 2, 0)
            for r in results]
    return np.concatenate(outs, axis=0)


def kernel(**inputs) -> np.ndarray:
    nc = get_nc()
    in_maps = make_in_maps(inputs)
    res = run_bass_kernel_spmd(nc, in_maps, core_ids=list(range(NCORES)))
    return assemble_out(res.results)


# revision 20
# speedup vs baseline: 1.0147x; 1.0147x over previous
"""AnomalyTransformer forward pass on 8 Trainium2 NeuronCores.

Data-parallel over batch: each core processes 32 of the 256 batch elements
through the full 3-layer transformer.

Precision strategy: the residual stream h and all projection weights on
the q/k path run in float32r (TF32-like TensorEngine mode, full throughput
at moving-dim >= 256); q/k chunk tiles, the value path and attention
output run in bf16. Softmax logits reach +-38 in layer 3, so an all-bf16
kernel amplifies rounding to ~1.5e-2 relative error; this mix lands at
~6e-3 against the f32 reference.

Layout strategy: the residual stream h is feature-major ([D, tokens], D
split over 4 partition-tiles of 128). Attention uses the scoresT
orientation (scoresT = khT.T @ qhT -> [l_k, l_q]) so softmax normalization
folds into the attention-value matmul via an appended ones-column on V
(column 64 of each head's 65-wide slot accumulates sum(exp)); the
per-token reciprocal is then a per-partition scalar multiply. One
[100, 512] PE-transpose per batch element brings the attention output back
to feature-major for the Wo projection.

Pipeline strategy (what makes PE stay busy):
  - V-projection PSUM is double-buffered (2 banks) so the per-batch V
    matmuls overlap the DVE copies into SBUF.
  - The per-batch attention loop is software-pipelined on PE:
    scores(b) -> vproj(b+1) -> transpose(b-1) -> attnV(b), hiding the
    ACT exp latency and the DVE normalize latency behind PE work.
  - The last batch's transpose and the Wo projection of chunk g are
    deferred into chunk g+1 (independent token slices), hiding the
    normalize/copy chain behind a full projection block.
  - Copies are balanced across ACT and DVE (GPSIMD has no PSUM port and
    every post-matmul op reads PSUM, so it cannot help).
  - FFN runs W1 one chunk ahead of W2, with the residual add split
    between DVE adds and PE identity-matmul accumulation + ACT copyback;
    the output projection is folded into the last layer's FFN loop.
  - The circular-conv token embedding ships x once ([55, 102] padded
    windows per batch) and applies the k=3 unfold as 3 shifted-view
    matmuls, cutting input DMA 3x.

The sigma/prior branch of the reference is dead code (never feeds the
output) and is skipped. Biases in the reference are all zeros and are
skipped.
"""

import sys
import os
for _p in ("/opt/trn_rl_repo", "/root/.axon_site/_ro/trn_rl_repo"):
    if os.path.isdir(_p) and _p not in sys.path:
        sys.path.insert(0, _p)

import math
import numpy as np
import ml_dtypes

import concourse.bass as bass
import concourse.tile as tile
from concourse import mybir
from concourse.bass_utils import run_bass_kernel_spmd
from contextlib import ExitStack

BF16 = mybir.dt.bfloat16
F32 = mybir.dt.float32
F32R = mybir.dt.float32r
AF = mybir.ActivationFunctionType
OP = mybir.AluOpType

# model dims
B, L, C, D, H, NL, DFF = 256, 100, 55, 512, 8, 3, 64
DK = D // H                      # 64
NCORES = 8
BL = B // NCORES                 # 32 batches per core
TOK = BL * L                     # 3200 tokens per core
TCH = 400                        # token chunk (4 batches)
NT = TOK // TCH                  # 8 chunks
CB = TCH // L                    # 4 batches per chunk
KT = D // 128                    # 4 contraction tiles
LP = L + 2                       # padded window for the k=3 circular conv


_NOSTRUCT = ("InstDrain", "InstNoOp", "InstEventSemaphore", "InstHalt")


def _legalize_waits(nc, maxw=1):
    """This container's walrus caps sync-waits at 1 per instruction; move
    extra waits onto preceding same-engine NOPs (one wait each)."""
    cnt = [0]
    for f in nc.m.functions:
        for blk in f.blocks:
            newlist = []
            changed = False
            for ins in blk.instructions:
                si = getattr(ins, "sync_info", None)
                lim = maxw
                if si is not None and si.on_wait and len(si.on_wait) > lim:
                    waits = list(si.on_wait)
                    extra, keep = waits[:-lim], waits[-lim:]
                    for i in range(0, len(extra), 1):
                        cnt[0] += 1
                        nop = mybir.InstNoOp(
                            name=f"I-ws-{cnt[0]}", ins=[], outs=[], engine=ins.engine
                        )
                        nop.sync_info = mybir.SyncInfo(
                            on_wait=extra[i:i + 1], on_update=[]
                        )
                        newlist.append(nop)
                    ins.sync_info = mybir.SyncInfo(
                        on_wait=keep, on_update=list(si.on_update)
                    )
                    changed = True
                newlist.append(ins)
            if changed:
                blk.instructions = newlist
    return nc


def _offset_ap(ap, extra_offset, dims):
    """AP at ap.offset + extra_offset (elements) with free dims `dims`
    ([[step, count], ...]), keeping ap's partition dim."""
    return bass.AP(tensor=ap.tensor, offset=ap.offset + extra_offset,
                   ap=[list(ap.ap[0])] + [list(d) for d in dims])


def build_nc():
    nc = bass.Bass()

    # ---- DRAM parameters (host-prepped) ----
    # xpad[c, t, b, l] : per-chunk circular-padded windows, [55, NT*CB*102]
    xpad_d = nc.declare_dram_parameter("xpad", [C, NT, CB, LP], F32R, isOutput=False)
    wemb_d = nc.declare_dram_parameter("wemb", [C, 3, D], F32R, isOutput=False)
    pe_d = nc.declare_dram_parameter("pe", [128, KT, L], F32, isOutput=False)
    wq_d = nc.declare_dram_parameter("wq", [128, NL, KT, D], F32R, isOutput=False)
    wk_d = nc.declare_dram_parameter("wk", [128, NL, KT, D], F32R, isOutput=False)
    wv_d = nc.declare_dram_parameter("wv", [128, NL, KT, D], F32R, isOutput=False)
    wo_d = nc.declare_dram_parameter("wo", [128, NL, KT, D], BF16, isOutput=False)
    w1_d = nc.declare_dram_parameter("w1", [128, NL, KT, DFF], F32R, isOutput=False)
    w2_d = nc.declare_dram_parameter("w2", [DFF, NL, KT, 128], F32R, isOutput=False)
    wout_d = nc.declare_dram_parameter("wout", [128, KT, C], F32R, isOutput=False)
    identb_d = nc.declare_dram_parameter("identb", [128, 128], BF16, isOutput=False)
    identr_d = nc.declare_dram_parameter("identr", [128, 128], F32R, isOutput=False)
    out_d = nc.declare_dram_parameter("out", [C, TOK], F32, isOutput=True)

    with tile.TileContext(nc) as tc, ExitStack() as stk:
        tc.race_detector_enabled = False
        singles = stk.enter_context(tc.tile_pool(name="singles", bufs=1))
        wp = stk.enter_context(tc.tile_pool(name="wp", bufs=2))
        xp = stk.enter_context(tc.tile_pool(name="xp", bufs=3))
        qp = stk.enter_context(tc.tile_pool(name="qp", bufs=2))
        kp = stk.enter_context(tc.tile_pool(name="kp", bufs=2))
        vp = stk.enter_context(tc.tile_pool(name="vp", bufs=2))
        expp = stk.enter_context(tc.tile_pool(name="expp", bufs=3))
        op_ = stk.enter_context(tc.tile_pool(name="op", bufs=3))
        rp = stk.enter_context(tc.tile_pool(name="rp", bufs=3))
        otp = stk.enter_context(tc.tile_pool(name="otp", bufs=2))
        yp = stk.enter_context(tc.tile_pool(name="yp", bufs=2))
        outp = stk.enter_context(tc.tile_pool(name="outp", bufs=2))
        # psum pools (8 banks total); the FFN W1 accumulator borrows the
        # attention-idle vpp pool
        pp = stk.enter_context(tc.tile_pool(name="pp", bufs=3, space="PSUM"))
        scp = stk.enter_context(tc.tile_pool(name="scp", bufs=3, space="PSUM"))
        vpp = stk.enter_context(tc.tile_pool(name="vpp", bufs=2, space="PSUM"))

        # ---- persistent SBUF ----
        wemb_sb = singles.tile([C, 3, D], F32R)
        pe_sb = singles.tile([128, KT, L], F32)
        wout_sb = singles.tile([128, KT, C], F32R)
        ident_b = singles.tile([128, 128], BF16)
        ident_r = singles.tile([128, 128], F32R)
        ones_b = singles.tile([128, 1], BF16)
        nc.vector.memset(ones_b[:, :], 1.0)
        h_sb = [singles.tile([128, TOK], F32R, name=f"h{k}") for k in range(KT)]
        xc = [xp.tile([C, CB, LP], F32R, name=f"xc{t}", tag="xc") for t in range(NT)]

        # DMA priority order: embedding weights + first chunks of x first,
        # then the layer-0 projection weights (interleaved with remaining x
        # chunks), then late-needed singles.
        nc.sync.dma_start(out=wemb_sb[:], in_=wemb_d[:])
        nc.sync.dma_start(out=xc[0][:], in_=xpad_d[:, 0])
        nc.sync.dma_start(out=pe_sb[:], in_=pe_d[:])
        nc.sync.dma_start(out=xc[1][:], in_=xpad_d[:, 1])
        nc.sync.dma_start(out=ident_b[:], in_=identb_d[:])
        nc.sync.dma_start(out=ident_r[:], in_=identr_d[:])
        for t in range(2, NT):
            nc.sync.dma_start(out=xc[t][:], in_=xpad_d[:, t])

        # ---- token embedding: circular conv as 3 shifted matmuls, + pe ----
        for t in range(NT):
            tsl = slice(t * TCH, (t + 1) * TCH)
            for m in range(KT):
                ps = pp.tile([128, 512], F32, tag="pp")
                for d in range(3):
                    # moving: xc[t][:, b, d : d+100] for the 4 batches
                    mv = _offset_ap(xc[t][:, 0, 0], d, [[LP, CB], [1, L]])
                    nc.tensor.matmul(ps[:, :TCH], wemb_sb[:, d, m * 128:(m + 1) * 128],
                                     mv, start=(d == 0), stop=(d == 2))
                pe_b = _offset_ap(pe_sb[:, m, :], 0, [[0, CB], [1, L]])
                nc.vector.tensor_tensor(
                    h_sb[m][:, tsl].rearrange("p (b x) -> p b x", x=L),
                    ps[:, :TCH].rearrange("p (b x) -> p b x", x=L),
                    pe_b, op=OP.add)

        nc.sync.dma_start(out=wout_sb[:], in_=wout_d[:])

        # ---- transformer layers ----
        n_layer_passes = int(os.environ.get("ANOM_LAYERS", str(NL)))
        for lp_i in range(n_layer_passes):
            l = lp_i % NL
            last_layer = lp_i == n_layer_passes - 1
            wq_l = wp.tile([128, KT, D], F32R, tag="wq")
            wk_l = wp.tile([128, KT, D], F32R, tag="wk")
            wv_l = wp.tile([128, KT, D], F32R, tag="wv")
            wo_l = wp.tile([128, KT, D], BF16, tag="wo")
            w1_l = wp.tile([128, KT, DFF], F32R, tag="w1")
            w2_l = wp.tile([DFF, KT, 128], F32R, tag="w2")
            for dst, src in ((wq_l, wq_d), (wk_l, wk_d), (wv_l, wv_d),
                             (wo_l, wo_d), (w1_l, w1_d), (w2_l, w2_d)):
                nc.sync.dma_start(out=dst[:], in_=src[:, l])

            # ---- attention pass over chunks ----
            # pend = (g, ot_all, transpose_o, 3): the last batch's transpose
            # and the whole Wo projection of chunk g are deferred into chunk
            # g+1 so the normalize/copy chain hides behind QKproj(g+1).
            pend = None

            def do_wo(g, ot_all):
                gsl = slice(g * TCH, (g + 1) * TCH)
                for m in range(KT):
                    msl = slice(m * 128, (m + 1) * 128)
                    ps = pp.tile([128, 512], F32, tag="pp")
                    for k in range(KT):
                        nc.tensor.matmul(ps[:, :TCH], wo_l[:, k, msl],
                                         ot_all[:, k, :],
                                         start=(k == 0), stop=(k == KT - 1))
                    nc.vector.tensor_tensor(h_sb[m][:, gsl], ps[:, :TCH],
                                            h_sb[m][:, gsl], op=OP.add)

            def flush_pend():
                g_p, ot_p, tr_p = pend
                tr_p(CB - 1)
                do_wo(g_p, ot_p)

            for g in range(NT):
                gsl = slice(g * TCH, (g + 1) * TCH)

                # V tile, token-major per batch, natural head order
                v_t = vp.tile([128, CB, D], BF16, tag="v")

                def vproj_mm(bi):
                    b = g * CB + bi
                    bsl = slice(b * L, (b + 1) * L)
                    ps = vpp.tile([128, 512], F32, tag="vps", name="vps")
                    for k in range(KT):
                        nc.tensor.matmul(ps[:L, :], h_sb[k][:, bsl],
                                         wv_l[:, k, :],
                                         start=(k == 0), stop=(k == KT - 1))
                    return ps

                def vcopy_half(bi, ps, half, eng):
                    hsl = slice(half * 256, (half + 1) * 256)
                    eng(v_t[:L, bi, hsl], ps[:L, hsl])

                # batch-0 V projection first: PE filler that covers the
                # chunk-boundary wait on the previous batch's normalize
                # (the first QK psum slot is freed by it)
                vps0 = vproj_mm(0)
                vcopy_half(0, vps0, 0, nc.vector.tensor_copy)
                vcopy_half(0, vps0, 1, nc.scalar.copy)

                # Q/K projections for this chunk, feature-major [D, TCH]
                qc = [qp.tile([128, TCH], BF16, name=f"qc{m}", tag=f"qc{m}")
                      for m in range(KT)]
                kc = [kp.tile([128, TCH], BF16, name=f"kc{m}", tag=f"kc{m}")
                      for m in range(KT)]
                for m in range(KT):
                    msl = slice(m * 128, (m + 1) * 128)
                    ps = pp.tile([128, 512], F32, tag="pp")
                    for k in range(KT):
                        nc.tensor.matmul(ps[:, :TCH], wq_l[:, k, msl],
                                         h_sb[k][:, gsl],
                                         start=(k == 0), stop=(k == KT - 1))
                    nc.scalar.copy(qc[m][:], ps[:, :TCH])
                for m in range(KT):
                    msl = slice(m * 128, (m + 1) * 128)
                    ps = pp.tile([128, 512], F32, tag="pp")
                    for k in range(KT):
                        nc.tensor.matmul(ps[:, :TCH], wk_l[:, k, msl],
                                         h_sb[k][:, gsl],
                                         start=(k == 0), stop=(k == KT - 1))
                    nc.scalar.copy(kc[m][:], ps[:, :TCH])

                # previous chunk's deferred transpose + Wo projection: keeps
                # PE fed while this chunk's attention pipeline fills, and
                # gives the previous chunk's normalize/copy chain time.
                if pend is not None:
                    flush_pend()
                    pend = None

                ot_all = otp.tile([128, KT, TCH], BF16, tag="ot")
                o_ts = [None] * CB

                def transpose_o(bi):
                    tp = pp.tile([128, 1024], BF16, tag="pp", name="tp")
                    for m in range(KT):
                        nc.tensor.transpose(tp[:, m * L:(m + 1) * L],
                                            o_ts[bi][:L, m * 128:(m + 1) * 128],
                                            ident_b[:L, :L])
                    nc.vector.tensor_copy(
                        _offset_ap(ot_all[:, :, :], bi * L, [[TCH, KT], [1, L]]),
                        tp[:, :KT * L].rearrange("p (m x) -> p m x", x=L))

                for bi in range(CB):
                    csl = slice(bi * L, (bi + 1) * L)
                    # scoresT for 8 heads: even heads -> scA, odd -> scB
                    # (different PE row groups must write different PSUM
                    # banks). Evens first so exp(scA) starts early.
                    scA = scp.tile([128, 512], F32, tag="sc")
                    scB = scp.tile([128, 512], F32, tag="sc")
                    for hh in (0, 2, 4, 6, 1, 3, 5, 7):
                        kt_i, base = divmod(hh * DK, 128)
                        sc = scA if hh % 2 == 0 else scB
                        col = (hh // 2) * 128
                        nc.tensor.matmul(sc[:L, col:col + L],
                                         kc[kt_i][base:base + DK, csl],
                                         qc[kt_i][base:base + DK, csl],
                                         start=True, stop=True)
                    vps_n = vproj_mm(bi + 1) if bi + 1 < CB else None
                    exp_t = expp.tile([128, 8 * L], BF16, tag="exp")
                    # exp; head hh lands at exp_t cols hh*L
                    nc.scalar.activation(
                        exp_t[:L, :].rearrange("p (h x) -> p h x", x=2 * L)[:, :, :L],
                        scA[:L, :].rearrange("p (h x) -> p h x", x=128)[:, :, :L],
                        AF.Exp)
                    nc.scalar.activation(
                        _offset_ap(exp_t[:L, :], L, [[2 * L, 4], [1, L]]),
                        scB[:L, :].rearrange("p (h x) -> p h x", x=128)[:, :, :L],
                        AF.Exp)
                    # V copy halves for the next batch: ACT half after the
                    # exps, DVE half ahead of the normalize chain
                    if vps_n is not None:
                        vcopy_half(bi + 1, vps_n, 1, nc.scalar.copy)
                        vcopy_half(bi + 1, vps_n, 0, nc.vector.tensor_copy)
                    # oU[:, 64h:64h+64] = expST(h).T @ v(h); sums[:, h] =
                    # expST(h).T @ 1 (1-column matmuls are ~free on PE and
                    # let ou fit one bank -> one recip + one normalize)
                    ou = pp.tile([128, 512], F32, tag="pp", name="ou")
                    sums = scp.tile([128, 512], F32, tag="sc", name="sums")
                    tp = (pp.tile([128, 1024], BF16, tag="pp", name="tp")
                          if bi > 0 else None)
                    for hh in (0, 2, 4, 6):
                        nc.tensor.matmul(ou[:L, hh * 64:hh * 64 + 64],
                                         exp_t[:L, hh * L:(hh + 1) * L],
                                         v_t[:L, bi, hh * 64:(hh + 1) * 64],
                                         start=True, stop=True)
                        nc.tensor.matmul(sums[:L, hh:hh + 1],
                                         exp_t[:L, hh * L:(hh + 1) * L],
                                         ones_b[:L, :],
                                         start=True, stop=True)
                    if bi > 0:
                        for m in range(KT):
                            nc.tensor.transpose(
                                tp[:, m * L:(m + 1) * L],
                                o_ts[bi - 1][:L, m * 128:(m + 1) * 128],
                                ident_b[:L, :L])
                    for hh in (1, 3, 5, 7):
                        nc.tensor.matmul(ou[:L, hh * 64:hh * 64 + 64],
                                         exp_t[:L, hh * L:(hh + 1) * L],
                                         v_t[:L, bi, hh * 64:(hh + 1) * 64],
                                         start=True, stop=True)
                        nc.tensor.matmul(sums[:L, hh:hh + 1],
                                         exp_t[:L, hh * L:(hh + 1) * L],
                                         ones_b[:L, :],
                                         start=True, stop=True)
                    r_t = rp.tile([128, 8], F32, tag="r")
                    o_t = op_.tile([128, D], BF16, tag="o")
                    nc.vector.reciprocal(r_t[:L, :8], sums[:L, :8])
                    nc.vector.tensor_tensor(
                        o_t[:L, :].rearrange("p (h x) -> p h x", x=64),
                        ou[:L, :].rearrange("p (h x) -> p h x", x=64),
                        r_t[:L, :8].rearrange(
                            "p (h x) -> p h x", x=1).broadcast_to([L, 8, 64]),
                        op=OP.mult)
                    if bi > 0:
                        nc.vector.tensor_copy(
                            _offset_ap(ot_all[:, :, :], (bi - 1) * L,
                                       [[TCH, KT], [1, L]]),
                            tp[:, :KT * L].rearrange("p (m x) -> p m x", x=L))
                    o_ts[bi] = o_t
                pend = (g, ot_all, transpose_o)
            flush_pend()

            # ---- FFN pass (W1 runs one chunk ahead of W2) ----
            ps1s = [None] * NT

            def ffn1(g):
                gsl = slice(g * TCH, (g + 1) * TCH)
                ps1 = vpp.tile([128, 512], F32, tag="vps", name="ps1")
                for k in range(KT):
                    nc.tensor.matmul(ps1[:DFF, :TCH], w1_l[:, k, :],
                                     h_sb[k][:, gsl],
                                     start=(k == 0), stop=(k == KT - 1))
                y_t = yp.tile([DFF, TCH], F32R, tag="y")
                nc.scalar.activation(y_t[:, :], ps1[:DFF, :TCH], AF.Gelu)
                return y_t

            def outproj(g):
                tsl = slice(g * TCH, (g + 1) * TCH)
                ps = pp.tile([128, 512], F32, tag="pp")
                for k in range(KT):
                    nc.tensor.matmul(ps[:C, :TCH], wout_sb[:, k, :],
                                     h_sb[k][:, tsl],
                                     start=(k == 0), stop=(k == KT - 1))
                o_f = outp.tile([128, TCH], F32, tag="outc")
                nc.scalar.copy(o_f[:C, :], ps[:C, :TCH])
                nc.sync.dma_start(out=out_d[:, tsl], in_=o_f[:C, :])

            ys = [None] * NT
            ys[0] = ffn1(0)
            for g in range(NT):
                gsl = slice(g * TCH, (g + 1) * TCH)
                if g + 1 < NT:
                    ys[g + 1] = ffn1(g + 1)
                for m in range(KT):
                    ps2 = pp.tile([128, 512], F32, tag="pp", name="ps2")
                    if m < 2:
                        nc.tensor.matmul(ps2[:, :TCH], w2_l[:, m, :], ys[g][:, :],
                                         start=True, stop=True)
                        nc.vector.tensor_tensor(h_sb[m][:, gsl], ps2[:, :TCH],
                                                h_sb[m][:, gsl], op=OP.add)
                    else:
                        # residual folded into PSUM as an identity matmul;
                        # copyback split ACT/DVE (balances the two engines)
                        nc.tensor.matmul(ps2[:, :TCH], w2_l[:, m, :], ys[g][:, :],
                                         start=True, stop=False)
                        nc.tensor.matmul(ps2[:, :TCH], ident_r[:],
                                         h_sb[m][:, gsl],
                                         start=False, stop=True)
                        (nc.scalar.copy if m == 2 else nc.vector.tensor_copy)(
                            h_sb[m][:, gsl], ps2[:, :TCH])
                if last_layer and g >= 1:
                    outproj(g - 1)
            if last_layer:
                outproj(NT - 1)

    return _legalize_waits(nc)


def _bf(a):
    return np.ascontiguousarray(a).astype(ml_dtypes.bfloat16)


def _r32(a):
    """Round to the reduced-dtype grid (f32r: 10 explicit mantissa bits)."""
    a = np.ascontiguousarray(a, np.float32)
    u = a.view(np.uint32).copy()
    u = (u + 0x1000) & 0xFFFFE000
    return u.view(np.float32)


def prep_weights(tok_w, pe, Wq, Wk, Wv, Wo, W1, W2, proj_w):
    """Host-side weight reorganization (shared across cores)."""
    scale = 1.0 / math.sqrt(DK)
    # conv as 3 shifted matmuls: wemb[c, d, o] = tok_w[o, c, d]
    wemb = np.ascontiguousarray(np.transpose(tok_w, (1, 2, 0)))  # [C, 3, D]
    # projection weights as lhsT tiles: w[p, l, k, j] = W[l, j, 128k + p]
    def proj_lhsT(W):  # [NL, D_out, D_in] -> [128, NL, KT, D_out]
        return np.ascontiguousarray(
            np.transpose(W, (2, 0, 1)).reshape(KT, 128, NL, W.shape[1])
            .transpose(1, 2, 0, 3))
    eye = np.eye(128, dtype=np.float32)
    m = {
        "identb": _bf(eye), "identr": _r32(eye),
        "wemb": _r32(wemb),
        "pe": np.ascontiguousarray(
            np.ascontiguousarray(pe.T).reshape(KT, 128, L).transpose(1, 0, 2)),
        "wq": _r32(proj_lhsT(Wq * scale)),
        "wk": _r32(proj_lhsT(Wk)),
        "wv": _r32(proj_lhsT(Wv)),
        "wo": _bf(proj_lhsT(Wo)),
        "w1": _r32(proj_lhsT(W1)),
        # w2[p, l, m, j] = W2[l, 128m + j, p]   (p over DFF=64)
        "w2": _r32(np.transpose(W2, (2, 0, 1)).reshape(DFF, NL, KT, 128)),
        # wout[p, k, j] = proj_w[j, 128k + p]
        "wout": _r32(np.ascontiguousarray(proj_w.T).reshape(KT, 128, C)
                     .transpose(1, 0, 2)),
    }
    return m


def prep_xpad(xs):
    """Per-core input: xs [BL, L, C] -> feature-major circular-padded
    windows [C, NT, CB, L+2]."""
    xt = np.transpose(xs, (2, 0, 1))                     # [C, BL, L]
    xpad = np.empty((C, BL, LP), np.float32)
    xpad[:, :, 1:L + 1] = xt
    xpad[:, :, 0] = xt[:, :, L - 1]
    xpad[:, :, L + 1] = xt[:, :, 0]
    return _r32(xpad.reshape(C, NT, CB, LP))


_NC_CACHE = {}


def get_nc():
    if "nc" not in _NC_CACHE:
        _NC_CACHE["nc"] = build_nc()
    return _NC_CACHE["nc"]


def make_in_maps(inputs):
    x = np.asarray(inputs["x"], np.float32)
    wm = prep_weights(np.asarray(inputs["tok_w"], np.float32),
                      np.asarray(inputs["pe"], np.float32),
                      np.asarray(inputs["Wq"], np.float32),
                      np.asarray(inputs["Wk"], np.float32),
                      np.asarray(inputs["Wv"], np.float32),
                      np.asarray(inputs["Wo"], np.float32),
                      np.asarray(inputs["W1"], np.float32),
                      np.asarray(inputs["W2"], np.float32),
                      np.asarray(inputs["proj_w"], np.float32))
    in_maps = []
    for c in range(NCORES):
        in_maps.append({**wm, "xpad": prep_xpad(x[c * BL:(c + 1) * BL])})
    return in_maps


def assemble_out(results):
    # per-core out [C, TOK] feature-major -> [B, L, C]
    outs = [np.asarray(r["out"], np.float32).reshape(C, BL, L).transpose(1, 2, 0)
            for r in results]
    return np.concatenate(outs, axis=0)


def kernel(**inputs) -> np.ndarray:
    nc = get_nc()
    in_maps = make_in_maps(inputs)
    res = run_bass_kernel_spmd(nc, in_maps, core_ids=list(range(NCORES)))
    return assemble_out(res.results)
